# revision 25
# baseline (speedup 1.0000x reference)
"""NeighborAttentionLayer Trainium2 kernel (8-core data-parallel SPMD).

Strategy
--------
Data-parallel over the batch dim B=64: each of the 8 NeuronCores runs the
full transformer layer for 8 batches (1024 tokens). No collectives.

The large projections (QKV, out_proj) run in fp8-e4m3 with
`perf_mode=DoubleRow` (contracting 256 rows per matmul, ~1.5-1.9x the bf16
matmul rate); attention core (scores/softmax/attn@V) and the FFN stay bf16
for the error budget. All fp8 scale factors are exact powers of two and are
folded into existing ops:
  * q/k descale rides the PSUM->SBUF activation copy's `scale`
  * v carries its scale into attn@V; the aoT fp8 store rescales on its copy
  * out_proj's carried scale is matched by pre-scaling the residual x on
    the host; LayerNorm (no affine) is exactly scale-invariant, so LN1
    absorbs the whole factor.

Host-side prep (numpy, not on HW):
  * weights transposed to [in_features, out_features], quantized (fp8 for
    qkv/out, bf16 for ffn), pre-tiled into per-tile-contiguous blocks
  * 1/sqrt(head_dim) folded into the q projection
  * q/k out-features permuted into a head-pair-interleaved order so every
    head's 320 features map onto 128-partition tiles as 128+128+64 slices
  * x shard passed both natural fp32 (residual, pre-scaled) and transposed
    fp8 (matmul operand)

The learned distance-bias MLP adds a per-query bias broadcast over keys;
softmax over keys is invariant to it, so it is skipped. The key-padding
mask is all-ones per the problem spec (fill=ones); a non-trivial mask is
applied multiplicatively on the exp'd scores.
"""

import numpy as np
import ml_dtypes

# ---- problem constants (hardcoded per contract) ----
B, K, D, H, DFF = 64, 128, 2560, 8, 1024
HD = D // H                    # 320
EPS = 1e-5
NCORES = 8
BL = B // NCORES               # 8 batches per core
TOK = BL * K                   # 1024 tokens per core
P = 128
DT = D // P                    # 20 d-tiles
FT = DFF // P                  # 8 dff-tiles
CH = 512                       # matmul moving-dim chunk (psum bank limit)
NHALF = 2                      # token halves for attention SBUF pressure
THALF = TOK // NHALF           # 512 tokens per half
BHALF = BL // NHALF            # 4 batches per half
QKT = 2 * DT                   # 40 q+k feature tiles

F8NP = ml_dtypes.float8_e4m3   # TRN FP8_EXP4-compatible (max 240)
AO_T = 5                       # log2 target scale for the fp8 aoT store


def _qk_perm():
    """Head-pair interleaved feature order for q (and k) projections."""
    perm = []
    for p in range(H // 2):
        h0, h1 = 2 * p, 2 * p + 1
        perm.extend(range(HD * h0, HD * h0 + 256))         # tiles 5p+0, 5p+1
        perm.extend(range(HD * h0 + 256, HD * h0 + 320))   # tile 5p+2 lo
        perm.extend(range(HD * h1 + 256, HD * h1 + 320))   # tile 5p+2 hi
        perm.extend(range(HD * h1, HD * h1 + 256))         # tiles 5p+3, 5p+4
    return np.array(perm)


def _score_ktiles(h):
    """(tile, row0, row1) triples (within the 20 q-tiles) contracting head h."""
    p = h // 2
    if h % 2 == 0:
        return [(5 * p + 0, 0, 128), (5 * p + 1, 0, 128), (5 * p + 2, 0, 64)]
    return [(5 * p + 3, 0, 128), (5 * p + 4, 0, 128), (5 * p + 2, 64, 128)]


def _ao_segments():
    """Per d-tile (real feature order) segments for attn@V:
    list over tiles of [(head, d0, d1, psum_base), ...]."""
    segs = [[] for _ in range(DT)]
    for h in range(H):
        d = HD * h
        end = HD * (h + 1)
        while d < end:
            nxt = min(end, (d // P + 1) * P)
            segs[d // P].append((h, d, nxt, d % P))
            d = nxt
    return segs


def _tileize(wT, chunk):
    """[Kin, N] -> [N/chunk, 128, Kin/128, chunk] contiguous blocks."""
    kin, n = wT.shape
    ko = kin // P
    return np.ascontiguousarray(
        wT.reshape(ko, P, n // chunk, chunk).transpose(2, 1, 0, 3))


def _q8(w):
    """Quantize to fp8-e4m3 with a power-of-two scale. Returns (q, exp)."""
    m = float(np.abs(w).max())
    e = int(np.floor(np.log2(224.0 / m)))
    s = np.float32(2.0 ** e)
    q = np.clip(w.astype(np.float32) * s, -240.0, 240.0).astype(F8NP)
    return q, e


def build_core_program(use_qk_bias, use_v_bias, use_out_bias, use_b1, use_b2,
                       ln1_affine, ln2_affine, use_mask, ao_desc):
    import concourse.bass as bass
    import concourse.bacc as bacc
    import concourse.mybir as mybir
    import concourse.tile as tile
    from concourse.masks import make_identity

    F32 = mybir.dt.float32
    BF16 = mybir.dt.bfloat16
    F8 = mybir.dt.float8e4
    DR = mybir.MatmulPerfMode.DoubleRow

    nc = bacc.Bacc()
    dp = nc.declare_dram_parameter
    xT = dp("xT", [NHALF, P, DT, THALF], F8, isOutput=False)
    xTb = dp("xTb", [NHALF, P, DT, THALF], BF16, isOutput=False)
    x_nat = dp("x", [TOK, D], F32, isOutput=False)
    qk_wT = dp("qk_wT", [QKT, P, DT, P], BF16, isOutput=False)
    v_wT = dp("v_wT", [D // CH, P, DT, CH], F8, isOutput=False)
    out_wT = dp("out_wT", [D // CH, P, DT, CH], F8, isOutput=False)
    w1T = dp("w1T", [FT, P, DT, P], BF16, isOutput=False)
    w2T = dp("w2T", [D // CH, P, FT, CH], BF16, isOutput=False)
    qk_b = dp("qk_b", [2 * D], F32, isOutput=False) if use_qk_bias else None
    v_b = dp("v_b", [D], F32, isOutput=False) if use_v_bias else None
    out_b = dp("out_b", [D], F32, isOutput=False) if use_out_bias else None
    b1 = dp("b1", [DFF], F32, isOutput=False) if use_b1 else None
    b2 = dp("b2", [D], F32, isOutput=False) if use_b2 else None
    ln1_g = dp("ln1_g", [D], F32, isOutput=False) if ln1_affine else None
    ln1_b = dp("ln1_b", [D], F32, isOutput=False) if ln1_affine else None
    ln2_g = dp("ln2_g", [D], F32, isOutput=False) if ln2_affine else None
    ln2_b = dp("ln2_b", [D], F32, isOutput=False) if ln2_affine else None
    mask_in = dp("mask", [BL, K], F32, isOutput=False) if use_mask else None
    out = dp("out", [TOK, D], F32, isOutput=True)

    x1_dram = nc.dram_tensor("x1_scratch", [TOK, D], BF16)
    aoT_dram = nc.dram_tensor("aoT_scratch", [BL, P, DT, P], F8)

    Exp = mybir.ActivationFunctionType.Exp
    Relu = mybir.ActivationFunctionType.Relu
    Sqrt = mybir.ActivationFunctionType.Sqrt
    Copy = mybir.ActivationFunctionType.Copy
    Ident = mybir.ActivationFunctionType.Identity
    AX = mybir.AxisListType.X
    OP = mybir.AluOpType

    def bcast_dram(ap, n_part=P):
        return bass.AP(tensor=ap.tensor, offset=ap.offset,
                       ap=[[0, n_part]] + list(ap.ap))

    ao_segs = _ao_segments()

    with tile.TileContext(nc) as tc:
        with (
            tc.tile_pool(name="consts", bufs=1) as consts,
        ):
            id_bf = consts.tile([P, P], BF16)
            make_identity(nc, id_bf)
            eps_sb = consts.tile([P, 1], F32)
            nc.vector.memset(eps_sb, EPS)

            # first V-weight chunk rides the front of the gpsimd DMA ring
            # so the first matmul is not behind the sync-ring burst
            wv0_pre = consts.tile([P, DT, CH], F8)
            nc.gpsimd.dma_start(out=wv0_pre, in_=v_wT[0])

            # whole out_proj weight resident in SBUF (51KB/partition);
            # its DMAs are emitted after half-0's projections so they do
            # not contend with the startup-critical x/weight loads
            wo_res = consts.tile([P, D // CH, DT, CH], F8)

            qkb_sb = None
            if use_qk_bias:
                qkb_sb = consts.tile([P, QKT], F32)
                nc.sync.dma_start(out=qkb_sb,
                                  in_=qk_b[:].rearrange("(t p) -> p t", p=P))
            vb_sb = None
            if use_v_bias:
                vb_sb = consts.tile([P, D], F32)
                nc.gpsimd.dma_start(out=vb_sb, in_=bcast_dram(v_b[:]))
            outb_sb = None
            if use_out_bias:
                outb_sb = consts.tile([P, D], F32)
                nc.gpsimd.dma_start(out=outb_sb, in_=bcast_dram(out_b[:]))
            b1_sb = None
            if use_b1:
                b1_sb = consts.tile([P, FT], F32)
                nc.sync.dma_start(out=b1_sb,
                                  in_=b1[:].rearrange("(t p) -> p t", p=P))
            b2_sb = None
            if use_b2:
                b2_sb = consts.tile([P, D], F32)
                nc.gpsimd.dma_start(out=b2_sb, in_=bcast_dram(b2[:]))
            ln1g_sb = ln1b_sb = ln2g_sb = ln2b_sb = None
            if ln1_affine:
                ln1g_sb = consts.tile([P, D], F32)
                nc.gpsimd.dma_start(out=ln1g_sb, in_=bcast_dram(ln1_g[:]))
                ln1b_sb = consts.tile([P, D], F32)
                nc.gpsimd.dma_start(out=ln1b_sb, in_=bcast_dram(ln1_b[:]))
            if ln2_affine:
                ln2g_sb = consts.tile([P, D], F32)
                nc.gpsimd.dma_start(out=ln2g_sb, in_=bcast_dram(ln2_g[:]))
                ln2b_sb = consts.tile([P, D], F32)
                nc.gpsimd.dma_start(out=ln2b_sb, in_=bcast_dram(ln2_b[:]))
            mask_sb = None
            if use_mask:
                mask_sb = consts.tile([P, BL, K], F32)
                nc.gpsimd.dma_start(
                    out=mask_sb, in_=bcast_dram(mask_in[:, :]))

            # ======== attention: both halves share one set of buffers ========
            with (
                tc.tile_pool(name="attn_sb", bufs=1) as asb,
                tc.tile_pool(name="aw", bufs=2) as aw,
                tc.tile_pool(name="bt", bufs=2) as bt,
            ):
                xT_sb = asb.tile([P, DT, THALF], F8)
                xTb_sb = asb.tile([P, DT, THALF], BF16)
                v_sb = asb.tile([P, BHALF, D], BF16)
                qkT_sb = asb.tile([P, QKT, THALF], BF16)

                for half in range(NHALF):
                    nc.sync.dma_start(out=xT_sb, in_=xT[half])

                    with tc.tile_pool(name=f"aps{half}", bufs=4,
                                      space="PSUM") as aps:
                        # V projection: natural [tok, vfeat] (carries scale)
                        for c in range(D // CH):
                            if half == 0 and c == 0:
                                wv = wv0_pre
                            else:
                                wv = aw.tile([P, DT, CH], F8, tag="wv")
                                nc.sync.dma_start(out=wv, in_=v_wT[c])
                            for t in range(BHALF):
                                ps = aps.tile([P, CH], F32, tag="ps_a")
                                for k in range(0, DT, 2):
                                    nc.tensor.matmul(
                                        ps, xT_sb[:, k:k + 2, t * P:(t + 1) * P],
                                        wv[:, k:k + 2, :],
                                        start=(k == 0), stop=(k == DT - 2),
                                        perf_mode=DR)
                                if use_v_bias:
                                    nc.vector.tensor_add(
                                        out=v_sb[:, t, c * CH:(c + 1) * CH],
                                        in0=ps,
                                        in1=vb_sb[:, c * CH:(c + 1) * CH])
                                else:
                                    nc.vector.tensor_copy(
                                        out=v_sb[:, t, c * CH:(c + 1) * CH],
                                        in_=ps)

                        # Q/K projection: transposed [feat, tok] (bf16 for
                        # score accuracy -- fp8 here fails the error budget)
                        nc.sync.dma_start(out=xTb_sb, in_=xTb[half])
                        for jt in range(QKT):
                            wq = aw.tile([P, DT, P], BF16, tag="wq")
                            nc.sync.dma_start(out=wq, in_=qk_wT[jt])
                            ps = aps.tile([P, CH], F32, tag="ps_a")
                            for k in range(DT):
                                nc.tensor.matmul(ps, wq[:, k, :],
                                                 xTb_sb[:, k, :],
                                                 start=(k == 0),
                                                 stop=(k == DT - 1))
                            if use_qk_bias:
                                nc.scalar.activation(
                                    out=qkT_sb[:, jt, :], in_=ps, func=Ident,
                                    bias=qkb_sb[:, jt:jt + 1], scale=1.0)
                            else:
                                nc.scalar.activation(out=qkT_sb[:, jt, :],
                                                     in_=ps, func=Copy)

                    if half == 0:
                        for c in range(D // CH):
                            nc.gpsimd.dma_start(out=wo_res[:, c],
                                                in_=out_wT[c])

                    # attention per batch: scores -> transposes -> attn@V,
                    # each stage contiguous on PE so no mid-stream waits
                    with (
                        tc.tile_pool(name=f"sps{half}", bufs=4,
                                     space="PSUM") as sps,
                        tc.tile_pool(name=f"tps{half}", bufs=2,
                                     space="PSUM") as tps,
                        tc.tile_pool(name=f"ops{half}", bufs=2,
                                     space="PSUM") as ops,
                    ):
                        for bi in range(BHALF):
                            b = half * BHALF + bi
                            csl = slice(bi * P, (bi + 1) * P)
                            attn = bt.tile([P, H, P], BF16, tag="attn")
                            esum = bt.tile([P, H], F32, tag="esum")
                            rinv = bt.tile([P, H], F32, tag="rinv")
                            attnT = bt.tile([P, H, P], BF16, tag="attnT")
                            scs = []
                            for h in range(H):
                                sc = sps.tile([P, P], F32, tag="sc")
                                scs.append(sc)
                                kts = _score_ktiles(h)
                                for i, (t, r0, r1) in enumerate(kts):
                                    nc.tensor.matmul(
                                        sc, qkT_sb[r0:r1, t, csl],
                                        qkT_sb[r0:r1, DT + t, csl],
                                        start=(i == 0), stop=(i == len(kts) - 1))
                                # scores are bounded (|s| < ~30): exp cannot
                                # overflow fp32, so skip the max-subtraction
                                nc.scalar.activation(
                                    out=attn[:, h, :], in_=sc, func=Exp,
                                    accum_out=esum[:, h:h + 1])
                                if use_mask:
                                    nc.vector.tensor_mul(
                                        out=attn[:, h, :], in0=attn[:, h, :],
                                        in1=mask_sb[:, b, :])
                                    nc.vector.tensor_reduce(
                                        out=esum[:, h:h + 1], in_=attn[:, h, :],
                                        axis=AX, op=OP.add)
                                nc.vector.reciprocal(out=rinv[:, h:h + 1],
                                                     in_=esum[:, h:h + 1])
                                nc.vector.tensor_scalar_mul(
                                    out=attn[:, h, :], in0=attn[:, h, :],
                                    scalar1=rinv[:, h:h + 1])
                            for h in range(H):
                                tp = tps.tile([P, P], F32, tag="tp")
                                nc.tensor.matmul(tp, attn[:, h, :], id_bf,
                                                 start=True, stop=True)
                                nc.vector.tensor_copy(out=attnT[:, h, :], in_=tp)
                            ao_stage = bt.tile([P, DT, P], F8, tag="ao_stage")
                            for t in range(DT):
                                ao = ops.tile([P, P], F32, tag="ao")
                                for (h, d0, d1, base) in ao_segs[t]:
                                    w = d1 - d0
                                    nc.tensor.matmul(
                                        ao[base:base + w, :], v_sb[:, bi, d0:d1],
                                        attnT[:, h, :], start=True, stop=True,
                                        tile_position=((0, base) if base
                                                       else None))
                                nc.scalar.activation(out=ao_stage[:, t, :],
                                                     in_=ao, func=Copy,
                                                     scale=ao_desc)
                            nc.sync.dma_start(out=aoT_dram[b], in_=ao_stage)

            # ======== out_proj + residual + LN1 + FFN1 ========
            # token-tile-outer with resident out_proj weights; LN1+transpose
            # of tile ti-1 is emitted after tile ti's matmuls so the PE never
            # waits on the LayerNorm vector chain.
            NGRP = 2
            TPG = BL // NGRP
            GW = TPG * P              # tokens per FFN1 group (512)
            with tc.tile_pool(name="hres", bufs=1) as hres:
                hT = hres.tile([P, FT, TOK], BF16)
                with (
                    tc.tile_pool(name="csb", bufs=2) as csb,
                    tc.tile_pool(name="cao", bufs=2) as cao,
                    tc.tile_pool(name="cy", bufs=2) as cy,
                    tc.tile_pool(name="cx1t", bufs=1) as cx1t,
                    tc.tile_pool(name="dw", bufs=3) as dw,
                    tc.tile_pool(name="cps", bufs=4, space="PSUM") as cps,
                    tc.tile_pool(name="ctps", bufs=2, space="PSUM") as ctps,
                ):
                    x1T_grp = cx1t.tile([P, DT, TOK], BF16, tag="x1T")
                    stats_all = csb.tile([P, BL, 5, 6], F32, tag="stats")
                    y_tiles = [None] * BL

                    def emit_ln1_transp(tt):
                        yt = y_tiles[tt]
                        mv = csb.tile([P, 2], F32, tag="mv")
                        nc.vector.bn_aggr(out=mv, in_=stats_all[:, tt])
                        std = csb.tile([P, 1], F32, tag="std")
                        nc.scalar.activation(out=std, in_=mv[:, 1:2],
                                             func=Sqrt, bias=eps_sb,
                                             scale=1.0)
                        rstd = csb.tile([P, 1], F32, tag="rstd")
                        nc.vector.reciprocal(out=rstd, in_=std)
                        x1_t = csb.tile([P, D], BF16, tag="x1t")
                        nc.vector.tensor_scalar(out=x1_t, in0=yt,
                                                scalar1=mv[:, 0:1],
                                                scalar2=rstd,
                                                op0=OP.subtract, op1=OP.mult)
                        if ln1_affine:
                            nc.vector.tensor_mul(out=x1_t, in0=x1_t,
                                                 in1=ln1g_sb)
                            nc.vector.tensor_add(out=x1_t, in0=x1_t,
                                                 in1=ln1b_sb)
                        nc.sync.dma_start(
                            out=x1_dram[tt * P:(tt + 1) * P, :], in_=x1_t)
                        # transpose via matmul against identity: N=128
                        # matmuls stream ~3x faster than transpose-mode
                        for k in range(DT):
                            tp = ctps.tile([P, P], F32, tag="tp_c")
                            nc.tensor.matmul(
                                tp, x1_t[:, k * P:(k + 1) * P], id_bf,
                                start=True, stop=True)
                            nc.scalar.activation(
                                out=x1T_grp[:, k, tt * P:(tt + 1) * P],
                                in_=tp, func=Copy)

                    def emit_ffn1(g):
                        for ft in range(FT):
                            w1 = dw.tile([P, DT, P], BF16, tag="w1")
                            nc.gpsimd.dma_start(out=w1, in_=w1T[ft])
                            ps = cps.tile([P, CH], F32, tag="ps")
                            for k in range(DT):
                                nc.tensor.matmul(
                                    ps, w1[:, k, :],
                                    x1T_grp[:, k, g * GW:(g + 1) * GW],
                                    start=(k == 0), stop=(k == DT - 1))
                            osl = slice(g * GW, (g + 1) * GW)
                            if use_b1:
                                nc.scalar.activation(
                                    out=hT[:, ft, osl], in_=ps, func=Relu,
                                    bias=b1_sb[:, ft:ft + 1], scale=1.0)
                            else:
                                nc.scalar.activation(out=hT[:, ft, osl],
                                                     in_=ps, func=Relu)

                    for tt in range(BL):
                        aoT_t = cao.tile([P, DT, P], F8, tag="aoT_t")
                        nc.sync.dma_start(out=aoT_t, in_=aoT_dram[tt])
                        y_t = cy.tile([P, D], F32, tag="y")
                        y_tiles[tt] = y_t
                        for c in range(D // CH):
                            ps = cps.tile([P, CH], F32, tag="ps")
                            for k in range(0, DT, 2):
                                nc.tensor.matmul(
                                    ps, aoT_t[:, k:k + 2, :],
                                    wo_res[:, c, k:k + 2, :],
                                    start=(k == 0), stop=(k == DT - 2),
                                    perf_mode=DR)
                            if use_out_bias:
                                nc.vector.tensor_add(
                                    out=ps, in0=ps,
                                    in1=outb_sb[:, c * CH:(c + 1) * CH])
                            xr = csb.tile([P, CH], F32, tag="xr")
                            nc.sync.dma_start(
                                out=xr,
                                in_=x_nat[tt * P:(tt + 1) * P,
                                          c * CH:(c + 1) * CH])
                            nc.vector.tensor_add(
                                out=y_t[:, c * CH:(c + 1) * CH],
                                in0=ps, in1=xr)
                            nc.vector.bn_stats(
                                out=stats_all[:, tt, c, :],
                                in_=y_t[:, c * CH:(c + 1) * CH])
                        if tt > 0:
                            emit_ln1_transp(tt - 1)
                        if tt == TPG:
                            emit_ffn1(0)
                    emit_ln1_transp(BL - 1)
                    emit_ffn1(1)

                # ======== FFN2 + residual + LN2, per token group ========
                with (
                    tc.tile_pool(name="esb", bufs=2) as esb,
                    tc.tile_pool(name="ey", bufs=2) as ey,
                    tc.tile_pool(name="ew", bufs=2) as ew,
                    tc.tile_pool(name="eps", bufs=6, space="PSUM") as epsp,
                ):
                    for g in range(NGRP):
                        y2 = ey.tile([P, TPG, D], F32, tag="y2")
                        stats_e = esb.tile([P, TPG, 5, 6], F32, tag="stats_e")

                        def emit_ln2(g, ti):
                            tt = g * TPG + ti
                            mv = esb.tile([P, 2], F32, tag="mv_e")
                            nc.vector.bn_aggr(out=mv, in_=stats_e[:, ti])
                            std = esb.tile([P, 1], F32, tag="std_e")
                            nc.scalar.activation(out=std, in_=mv[:, 1:2],
                                                 func=Sqrt, bias=eps_sb,
                                                 scale=1.0)
                            rstd = esb.tile([P, 1], F32, tag="rstd_e")
                            nc.vector.reciprocal(out=rstd, in_=std)
                            o_t = esb.tile([P, D], F32, tag="o_t")
                            nc.vector.tensor_scalar(out=o_t, in0=y2[:, ti, :],
                                                    scalar1=mv[:, 0:1],
                                                    scalar2=rstd,
                                                    op0=OP.subtract,
                                                    op1=OP.mult)
                            if ln2_affine:
                                nc.vector.tensor_mul(out=o_t, in0=o_t,
                                                     in1=ln2g_sb)
                                nc.vector.tensor_add(out=o_t, in0=o_t,
                                                     in1=ln2b_sb)
                            nc.sync.dma_start(
                                out=out[tt * P:(tt + 1) * P, :], in_=o_t)

                        for c in range(D // CH):
                            w2c = ew.tile([P, FT, CH], BF16, tag="w2c")
                            nc.gpsimd.dma_start(out=w2c, in_=w2T[c])
                            for ti in range(TPG):
                                tt = g * TPG + ti
                                ps = epsp.tile([P, CH], F32, tag="ps_e")
                                for k in range(FT):
                                    nc.tensor.matmul(
                                        ps, hT[:, k, tt * P:(tt + 1) * P],
                                        w2c[:, k, :],
                                        start=(k == 0), stop=(k == FT - 1))
                                if use_b2:
                                    nc.vector.tensor_add(
                                        out=ps, in0=ps,
                                        in1=b2_sb[:, c * CH:(c + 1) * CH])
                                xr = esb.tile([P, CH], BF16, tag="xr_e")
                                nc.sync.dma_start(
                                    out=xr,
                                    in_=x1_dram[tt * P:(tt + 1) * P,
                                                c * CH:(c + 1) * CH])
                                nc.vector.tensor_add(
                                    out=y2[:, ti, c * CH:(c + 1) * CH],
                                    in0=ps, in1=xr)
                                nc.vector.bn_stats(
                                    out=stats_e[:, ti, c, :],
                                    in_=y2[:, ti, c * CH:(c + 1) * CH])
                                # group 0: LN2 one tile behind so psum-
                                # freeing adds reach the DVE queue first;
                                # last group: immediate, so no LayerNorm
                                # work remains after the final matmul
                                if c == D // CH - 1:
                                    if g == NGRP - 1:
                                        emit_ln2(g, ti)
                                    elif ti > 0:
                                        emit_ln2(g, ti - 1)
                        if g < NGRP - 1:
                            emit_ln2(g, TPG - 1)

    nc.compile()
    return nc


def _prep_inputs(x, distances, mask, qkv_w, qkv_b, out_w, out_b,
                 bias_w1, bias_b1, bias_w2, bias_b2,
                 ffn_w1, ffn_b1, ffn_w2, ffn_b2,
                 ln1_g, ln1_b, ln2_g, ln2_b):
    """Host-side shard + weight formatting. Returns (flags, in_maps)."""
    bf16 = ml_dtypes.bfloat16
    perm = _qk_perm()

    q_w = qkv_w[0:D][perm] * np.float32(1.0 / np.sqrt(HD))
    k_w = qkv_w[D:2 * D][perm]
    v_w = qkv_w[2 * D:3 * D]

    v_q, ev = _q8(v_w)
    o_q, eo = _q8(out_w)
    x_q, ex = _q8(x)

    qk_wT = _tileize(np.concatenate([q_w, k_w], axis=0).T.astype(bf16), P)
    v_wT = _tileize(v_q.T, CH)
    out_wT = _tileize(o_q.T, CH)
    w1T = _tileize(ffn_w1.T.astype(bf16), P)
    w2T = _tileize(ffn_w2.T.astype(bf16), CH)

    # scale bookkeeping (all powers of two, see module docstring)
    ao_desc = 2.0 ** (AO_T - (ex + ev))    # aoT carries 2**AO_T
    c_out = np.float32(2.0 ** (AO_T + eo))  # out_proj PSUM carried scale

    qk_b = np.concatenate([qkv_b[0:D][perm] * np.float32(1.0 / np.sqrt(HD)),
                           qkv_b[D:2 * D][perm]]).astype(np.float32)
    # v bias joins v's carried scale; out bias joins out_proj's carry
    v_b = (qkv_b[2 * D:3 * D] * np.float32(2.0 ** (ex + ev))).astype(np.float32)

    flags = dict(
        use_qk_bias=bool(np.any(qk_b != 0)),
        use_v_bias=bool(np.any(v_b != 0)),
        use_out_bias=bool(np.any(out_b != 0)),
        use_b1=bool(np.any(ffn_b1 != 0)),
        use_b2=bool(np.any(ffn_b2 != 0)),
        ln1_affine=not (np.all(ln1_g == 1) and np.all(ln1_b == 0)),
        ln2_affine=not (np.all(ln2_g == 1) and np.all(ln2_b == 0)),
        use_mask=not bool(np.all(mask)),
        ao_desc=ao_desc,
    )

    shared = {"qk_wT": qk_wT, "v_wT": v_wT, "out_wT": out_wT,
              "w1T": w1T, "w2T": w2T}
    if flags["use_qk_bias"]:
        shared["qk_b"] = qk_b
    if flags["use_v_bias"]:
        shared["v_b"] = v_b
    if flags["use_out_bias"]:
        shared["out_b"] = (out_b * c_out).astype(np.float32)
    if flags["use_b1"]:
        shared["b1"] = ffn_b1.astype(np.float32)
    if flags["use_b2"]:
        shared["b2"] = ffn_b2.astype(np.float32)
    if flags["ln1_affine"]:
        shared["ln1_g"] = ln1_g.astype(np.float32)
        shared["ln1_b"] = ln1_b.astype(np.float32)
    if flags["ln2_affine"]:
        shared["ln2_g"] = ln2_g.astype(np.float32)
        shared["ln2_b"] = ln2_b.astype(np.float32)

    in_maps = []
    for c in range(NCORES):
        xc = np.ascontiguousarray(
            x[c * BL:(c + 1) * BL].reshape(TOK, D)).astype(np.float32)
        xcq = np.ascontiguousarray(
            x_q[c * BL:(c + 1) * BL].reshape(TOK, D))
        xT_blocks = np.ascontiguousarray(
            xcq.T.reshape(DT, P, NHALF, THALF).transpose(2, 1, 0, 3))
        xTb_blocks = np.ascontiguousarray(
            xc.T.astype(bf16).reshape(DT, P, NHALF, THALF).transpose(2, 1, 0, 3))
        # residual pre-scaled to match out_proj's carried fp8 scale; LN1
        # (no affine) is exactly scale-invariant so this cancels there.
        m = {"x": xc * c_out, "xT": xT_blocks, "xTb": xTb_blocks, **shared}
        if flags["use_mask"]:
            m["mask"] = mask[c * BL:(c + 1) * BL].astype(np.float32)
        in_maps.append(m)
    return flags, in_maps


def run(trace=False, **inputs):
    """Build + run on 8 cores. Returns (output, BassKernelResults)."""
    from concourse.bass_utils import run_bass_kernel_spmd

    inputs = {k: np.asarray(v) for k, v in inputs.items()}
    flags, in_maps = _prep_inputs(**inputs)
    nc = build_core_program(**flags)
    res = run_bass_kernel_spmd(nc, in_maps, list(range(NCORES)), trace=trace)
    out = np.stack([np.asarray(res.results[c]["out"], dtype=np.float32)
                    for c in range(NCORES)])
    return out.reshape(B, K, D), res


def kernel(**inputs):
    out, _ = run(trace=False, **inputs)
    return out
